# revision 4
# baseline (speedup 1.0000x reference)
"""DeltaNet fused kernel for 8 TRN2 NeuronCores (Bass/Tile), fp8-hybrid v3.

Math (reference, with W_fast_w == 0 so v_bar == W_fast_b):
    s  = x @ W_slow_w.T + W_slow_b            [B, 3073]
    k  = s[:, :1024]; v = s[:, 1024:2048]; q = s[:, 2048:3072]
    lr = sigmoid(s[:, 3072])
    delta[o,h] = sum_b (lr*(v - wfb))[b,o] * sigmoid(k)[b,h] / B
    out = softmax(q) @ delta.T + wfb

Restructured to eliminate the v projection (v = x @ Wv.T + bv):
    g  = lr * sigmoid(k)                      [B, H]
    M  = x.T @ g                              [I, H]   (per-core partial)
    r  = sum_b g[b, :]                        [H]
    delta.T = (M.T @ Wv.T + r x (bv - wfb)) / B        [H, O]  (AllReduced)
    out = softmax(q) @ delta.T + wfb

v3 changes vs v2 (275us):
  * k contraction fully fp8-DR (validated: rel err unchanged at 1.11e-2 sim);
    drops wk16 + the 64 bf16 k matmuls.
  * delta.T is split by H-ROWS (not O-columns): pd half A only needs the
    h<512 part of g/M, so AR-A triggers at ~80us instead of ~177us and both
    AllReduces hide under k-B/M-B/pd-B + the q chunks.
  * lr multiply (g8 = sgk * lr) moved from Scalar to Pool so the k drain is
    one V add + one S sigmoid + one Pool mul -- no engine exceeds PE pace.
  * Host-side pre-tiled contiguous DMA layouts: one dma_start per big
    tensor with 4-16KB per-partition rows (was ~858B descriptors at
    ~206GB/s aggregate).
  * fin accumulates over both wn halves (h-contraction), runs last.
"""

import os
import sys

for _p in ("/opt/trn_rl_repo", "/root/.axon_site/_ro/trn_rl_repo"):
    if os.path.isdir(_p) and _p not in sys.path:
        sys.path.append(_p)

import numpy as np
import ml_dtypes

BF16 = ml_dtypes.bfloat16
F8E4 = ml_dtypes.float8_e4m3     # TRN fp8e4: max normal +-240

N_CORES = 8
B_FULL = 16384
DIM = 1024          # dim_in == dim_out == dim_hidden
P = 128
NT = DIM // P       # 8 tiles along any 1024 dim

SX = 16.0           # x fp8 scale             (|x|max 5.4  -> 87)
SWQ = 512.0         # Wq fp8 scale            (|w|max .16  -> 80)
SWK = 512.0         # Wk fp8 scale
SKP = SX * SWK      # k psum scale
SWL = 512.0         # wlr fp8 scale (i < 512 half)
SG = 128.0          # g fp8 scale             (g in (0,1)  -> <128)
SAR = 4096.0        # delta fp8 scale         (|delta|max .043 -> 176)
SHIFT = 3.0         # exp shift               (max q+bq 7.63 -> et < 103)


def _build_program(b_core: int, n_cores: int = N_CORES):
    """Build the SPMD Bass program (same program on every core)."""
    import concourse.bass as bass
    import concourse.mybir as mybir
    import concourse.tile as tile
    from concourse import bacc

    f32 = mybir.dt.float32
    bf16 = mybir.dt.bfloat16
    f8 = mybir.dt.float8e4
    AF = mybir.ActivationFunctionType
    ALU = mybir.AluOpType
    DR = mybir.MatmulPerfMode.DoubleRow

    nbt = b_core // P               # b-tiles per core (16)
    nbc = b_core // 512             # 512-wide b-chunks (4)
    nct = 512 // P                  # b-tiles per chunk (4)
    assert b_core % 1024 == 0

    nc = bacc.Bacc(
        "TRN2",
        target_bir_lowering=False,
        debug=False,
        num_devices=n_cores,
    )

    # ---- kernel I/O (host pre-tiled, fully contiguous) ----
    xT8a_h = nc.dram_tensor("xT8a", [P, NT * (b_core // 2)], f8, kind="ExternalInput")
    xT8b_h = nc.dram_tensor("xT8b", [P, NT * (b_core // 2)], f8, kind="ExternalInput")
    xn8_h = nc.dram_tensor("xn8", [P, nbt * DIM], f8, kind="ExternalInput")
    xT16_h = nc.dram_tensor("xT16", [P, 4 * b_core], bf16, kind="ExternalInput")
    wq8_h = nc.dram_tensor("wq8", [P, NT * DIM], f8, kind="ExternalInput")
    wk8A_h = nc.dram_tensor("wk8A", [P, NT * 512], f8, kind="ExternalInput")
    wk8B_h = nc.dram_tensor("wk8B", [P, NT * 512], f8, kind="ExternalInput")
    wv16_h = nc.dram_tensor("wv16", [P, NT * DIM], bf16, kind="ExternalInput")
    wlr8_h = nc.dram_tensor("wlr8", [512], f8, kind="ExternalInput")
    wlr16_h = nc.dram_tensor("wlr16", [512], bf16, kind="ExternalInput")
    bk_h = nc.dram_tensor("bk", [DIM], f32, kind="ExternalInput")     # bk * SKP
    bqs_h = nc.dram_tensor("bqs", [DIM], f32, kind="ExternalInput")   # bq - SHIFT
    blr_h = nc.dram_tensor("blr", [1], f32, kind="ExternalInput")
    # (bv - wfb) * SAR / (b_total * SG): outer-product operand for the drain
    bvcp_h = nc.dram_tensor("bvcp", [DIM], f32, kind="ExternalInput")
    wfb_h = nc.dram_tensor("wfb", [DIM], f32, kind="ExternalInput")
    out_h = nc.dram_tensor("out", [b_core, DIM], bf16, kind="ExternalOutput")

    pd_drain_scale = SAR / (float(b_core * n_cores) * SX * SG)
    fin_recip_scale = 1.0 / SAR
    q_act_scale = 1.0 / (SX * SWQ)
    k_act_scale = 1.0 / SKP
    lr_act_scale = 1.0 / (SX * SWL)

    with tile.TileContext(nc) as tc:
        with (
            tc.tile_pool(name="persist", bufs=1) as persist,
            tc.tile_pool(name="psum", bufs=6, space="PSUM") as psum,
            tc.tile_pool(name="psmall", bufs=2, space="PSUM") as psmall,
            tc.tile_pool(name="tmp", bufs=4) as tmp,
            tc.tile_pool(name="sgt", bufs=4) as sgt,
            tc.tile_pool(name="ost", bufs=4) as ost,
            tc.tile_pool(name="arst", bufs=3) as arst,
            tc.tile_pool(name="dram", bufs=1, space="DRAM") as dram,
        ):
            # ---- persistent SBUF tensors ----
            wq8 = persist.tile([P, NT, DIM], f8, name="wq8")
            xT8a = persist.tile([P, NT, b_core // 2], f8, name="xT8a")
            xT8b = persist.tile([P, NT, b_core // 2], f8, name="xT8b")
            wk8A = persist.tile([P, NT, 512], f8, name="wk8A")
            wk8B = persist.tile([P, NT, 512], f8, name="wk8B")
            # [P, 4, 16] with data in col 0: DR LDWEIGHTS needs pair step %16==0
            wlr8 = persist.tile([P, 4, 16], f8, name="wlr8")
            wlr16 = persist.tile([P, 4, 1], bf16, name="wlr16")
            xT16 = persist.tile([P, 4, b_core], bf16, name="xT16")
            xn8 = persist.tile([P, nbt, DIM], f8, name="xn8")
            wv16 = persist.tile([P, NT, DIM], bf16, name="wv16")
            g8 = persist.tile([P, nbt, DIM], f8, name="g8")
            et8 = persist.tile([P, NT, b_core], f8, name="et8")
            mb = persist.tile([P, NT, DIM], bf16, name="mb")
            wn8a = persist.tile([P, 4, DIM], f8, name="wn8a")
            wn8b = persist.tile([P, 4, DIM], f8, name="wn8b")
            bk_b = persist.tile([P, DIM], f32, name="bk_b")
            wfb_b = persist.tile([P, DIM], f32, name="wfb_b")
            bvcp_b = persist.tile([P, DIM], f32, name="bvcp_b")
            bq_c = persist.tile([P, NT], f32, name="bq_c")
            blr_c = persist.tile([P, 1], f32, name="blr_c")
            lr_c = persist.tile([P, nbt], f32, name="lr_c")
            r_c = persist.tile([P, NT], f32, name="r_c")
            recip_c = persist.tile([P, nbt], f32, name="recip_c")
            lrT_sb = persist.tile([1, b_core], f32, name="lrT_sb")
            rT_sb = persist.tile([1, DIM], f32, name="rT_sb")
            prsT_sb = persist.tile([1, b_core], f32, name="prsT_sb")
            ones8 = persist.tile([P, 2, 16], f8, name="ones8")
            ones_row = persist.tile([1, P], f32, name="ones_row")

            # ---- DRAM: AllReduce bounce (h-row halves) + transpose scratch ----
            ar_inA = dram.tile([512, DIM], f8, name="ar_inA")
            ar_inB = dram.tile([512, DIM], f8, name="ar_inB")
            ar_outA = dram.tile([512, DIM], f8, name="ar_outA", addr_space="Shared")
            ar_outB = dram.tile([512, DIM], f8, name="ar_outB", addr_space="Shared")
            sc_lr = dram.tile([nbt, P], f32, name="sc_lr")
            sc_r = dram.tile([NT, P], f32, name="sc_r")
            sc_prs = dram.tile([nbt, P], f32, name="sc_prs")
            warm_in = dram.tile([P, 512], f8, name="warm_in")
            warm_out = dram.tile([P, 512], f8, name="warm_out", addr_space="Shared")

            nc.vector.memset(ones8[:], 1.0)
            nc.vector.memset(ones_row[:], 1.0)

            # warm up the collective stream: the first collective pays a
            # ~50us barrier/setup cost; burn it on a tiny AllReduce that
            # runs concurrently with the startup DMAs.
            wtmp = tmp.tile([P, 512], f8, tag="kv", name="wtmp")
            nc.vector.memset(wtmp[:], 0.0)
            nc.gpsimd.dma_start(out=warm_in[:, :], in_=wtmp[:])
            nc.gpsimd.collective_compute(
                "AllReduce",
                mybir.AluOpType.add,
                replica_groups=[list(range(n_cores))],
                ins=[warm_in[:, :]],
                outs=[warm_out[:, :]],
            )

            # ---- small DMAs (gpsimd queue) ----
            nc.gpsimd.dma_start(
                out=bq_c[:],
                in_=bass.AP(tensor=bqs_h, offset=0, ap=[[1, P], [P, NT]]),
            )
            nc.gpsimd.dma_start(
                out=blr_c[:],
                in_=bass.AP(tensor=blr_h, offset=0, ap=[[0, P], [1, 1]]),
            )
            for i in range(4):
                nc.gpsimd.dma_start(
                    out=wlr8[:, i, 0:1],
                    in_=bass.AP(tensor=wlr8_h, offset=i * P, ap=[[1, P], [P, 1]]),
                )
                nc.gpsimd.dma_start(
                    out=wlr16[:, i, :],
                    in_=bass.AP(tensor=wlr16_h, offset=i * P, ap=[[1, P], [P, 1]]),
                )
            # bias broadcasts across partitions via K=1 ones-matmuls
            for bi, (bias_dst, bias_src) in enumerate(
                ((bk_b, bk_h), (wfb_b, wfb_h), (bvcp_b, bvcp_h))
            ):
                for c in range(2):
                    brow = tmp.tile([1, 512], f32, tag="br", name=f"br{bi}_{c}")
                    nc.gpsimd.dma_start(
                        out=brow[:],
                        in_=bass.AP(tensor=bias_src, offset=c * 512,
                                    ap=[[0, 1], [1, 512]]),
                    )
                    pb = psum.tile([P, 512], f32, tag="ps", name=f"pb{bi}_{c}")
                    nc.tensor.matmul(
                        pb[:], ones_row[:, :], brow[:], start=True, stop=True
                    )
                    nc.vector.tensor_copy(bias_dst[:, c * 512:(c + 1) * 512], pb[:])

            # ---- bulk DMAs: single contiguous transfers per tensor ----
            # sync queue: q/k stream operands then natural-layout x
            nc.sync.dma_start(out=xT8a[:, :, :], in_=xT8a_h[:, :])
            nc.sync.dma_start(out=wk8A[:, :, :], in_=wk8A_h[:, :])
            nc.sync.dma_start(
                out=xn8[:, 0:nbt // 2, :], in_=xn8_h[:, 0:(nbt // 2) * DIM]
            )
            nc.sync.dma_start(out=xT8b[:, :, :], in_=xT8b_h[:, :])
            nc.sync.dma_start(out=wk8B[:, :, :], in_=wk8B_h[:, :])
            nc.sync.dma_start(
                out=xn8[:, nbt // 2:, :], in_=xn8_h[:, (nbt // 2) * DIM:]
            )
            # scalar queue: wq8 first (q0 warmup), then lr operand, then wv
            nc.scalar.dma_start(out=wq8[:, :, :], in_=wq8_h[:, :])
            nc.scalar.dma_start(
                out=xT16[:, 0:2, :], in_=xT16_h[:, 0:2 * b_core]
            )
            nc.scalar.dma_start(
                out=xT16[:, 2:4, :], in_=xT16_h[:, 2 * b_core:]
            )
            nc.scalar.dma_start(out=wv16[:, :, :], in_=wv16_h[:, :])

            def xt8_lhs(t, j2):
                """fp8 xT lhsT pair slice for global b-tile t, i-pair j2."""
                src = xT8a if t < nbt // 2 else xT8b
                tc_ = t % (nbt // 2)
                return src[:, 2 * j2:2 * j2 + 2, tc_ * P:(tc_ + 1) * P]

            def emit_q(chunks):
                """et8 = exp(qT + bq - SHIFT), transposed layout [h, b]. fp8 DR."""
                for bc in chunks:
                    src = xT8a if bc < nbc // 2 else xT8b
                    lo = (bc % (nbc // 2)) * 512
                    for hb in range(NT):
                        pq = psum.tile([P, 512], f32, tag="ps", name=f"pq{bc}_{hb}")
                        for j in range(NT // 2):
                            nc.tensor.matmul(
                                pq[:],
                                wq8[:, 2 * j:2 * j + 2, hb * P:(hb + 1) * P],
                                src[:, 2 * j:2 * j + 2, lo:lo + 512],
                                start=(j == 0), stop=(j == NT // 2 - 1),
                                perf_mode=DR,
                            )
                        nc.scalar.activation(
                            et8[:, hb, bc * 512:(bc + 1) * 512], pq[:], AF.Exp,
                            bias=bq_c[:, hb:hb + 1], scale=q_act_scale,
                        )

            def emit_lrT(bc):
                """lr_c[p, 4bc+j] = sigmoid(x @ wlr + blr) * SG for one
                512-col chunk, via a transposed [1, 512] matmul + bounce."""
                src8 = xT8a if bc < nbc // 2 else xT8b
                lo = (bc % (nbc // 2)) * 512
                pl = psmall.tile([1, 512], f32, tag="pl", name=f"plr{bc}")
                for m in range(2):
                    nc.tensor.matmul(
                        pl[:],
                        wlr8[:, 2 * m:2 * m + 2, 0:1],
                        src8[:, 2 * m:2 * m + 2, lo:lo + 512],
                        start=(m == 0), stop=False,
                        perf_mode=DR,
                    )
                for i in range(4):
                    nc.tensor.matmul(
                        pl[:],
                        wlr16[:, i, 0:1],
                        xT16[:, i, bc * 512:(bc + 1) * 512],
                        start=False, stop=(i == 3),
                    )
                nc.scalar.activation(
                    lrT_sb[0:1, bc * 512:(bc + 1) * 512], pl[:], AF.Sigmoid,
                    bias=blr_c[0:1, 0:1], scale=lr_act_scale,
                )
                nc.gpsimd.dma_start(
                    out=sc_lr[nct * bc:nct * (bc + 1), :],
                    in_=lrT_sb[0:1, bc * 512:(bc + 1) * 512],
                )
                nc.gpsimd.dma_start(
                    out=lr_c[:, nct * bc:nct * (bc + 1)],
                    in_=sc_lr[nct * bc:nct * (bc + 1), :].rearrange("a b -> b a"),
                )
                nc.vector.tensor_scalar_mul(
                    lr_c[:, nct * bc:nct * (bc + 1)],
                    lr_c[:, nct * bc:nct * (bc + 1)], SG,
                )

            def emit_k(tiles, hc):
                """g8[:, t, hc half] = lr * sigmoid(k), natural layout [b, h].
                Contraction fully fp8-DR. Drain: V add bias, S sigmoid,
                Pool multiply by lr."""
                wk = wk8A if hc == 0 else wk8B
                for t in tiles:
                    pk = psum.tile([P, 512], f32, tag="ps", name=f"pk{t}_{hc}")
                    for j2 in range(NT // 2):
                        nc.tensor.matmul(
                            pk[:],
                            xt8_lhs(t, j2),
                            wk[:, 2 * j2:2 * j2 + 2, :],
                            start=(j2 == 0), stop=(j2 == NT // 2 - 1),
                            perf_mode=DR,
                        )
                    ktmp = tmp.tile([P, 512], f32, tag="kv", name=f"kt{t}_{hc}")
                    nc.vector.tensor_add(
                        ktmp[:], pk[:], bk_b[:, hc * 512:(hc + 1) * 512]
                    )
                    sgk = sgt.tile([P, 512], bf16, tag="sg", name=f"sg{t}_{hc}")
                    nc.scalar.activation(sgk[:], ktmp[:], AF.Sigmoid,
                                         scale=k_act_scale)
                    nc.gpsimd.tensor_scalar_mul(
                        g8[:, t, hc * 512:(hc + 1) * 512], sgk[:],
                        lr_c[:, t:t + 1],
                    )

            def emit_m(hc):
                """mb[:, :, hc half] = x.T @ g (per-core partial), fp8 DR;
                then rT half = ones.T @ g via [1, 512] matmuls + bounce."""
                for ib in range(NT):
                    pm = psum.tile([P, 512], f32, tag="ps", name=f"pm{hc}_{ib}")
                    for bp in range(nbt // 2):
                        nc.tensor.matmul(
                            pm[:],
                            xn8[:, 2 * bp:2 * bp + 2, ib * P:(ib + 1) * P],
                            g8[:, 2 * bp:2 * bp + 2, hc * 512:(hc + 1) * 512],
                            start=(bp == 0), stop=(bp == nbt // 2 - 1),
                            perf_mode=DR,
                        )
                    nc.vector.tensor_copy(
                        mb[:, ib, hc * 512:(hc + 1) * 512], pm[:]
                    )
                pr = psmall.tile([1, 512], f32, tag="pl", name=f"pr{hc}")
                for bp in range(nbt // 2):
                    nc.tensor.matmul(
                        pr[:],
                        ones8[:, 0:2, 0:1],
                        g8[:, 2 * bp:2 * bp + 2, hc * 512:(hc + 1) * 512],
                        start=(bp == 0), stop=(bp == nbt // 2 - 1),
                        perf_mode=DR,
                    )
                nc.vector.tensor_copy(
                    rT_sb[0:1, hc * 512:(hc + 1) * 512], pr[:]
                )
                nc.gpsimd.dma_start(
                    out=sc_r[nct * hc:nct * (hc + 1), :],
                    in_=rT_sb[0:1, hc * 512:(hc + 1) * 512],
                )
                nc.gpsimd.dma_start(
                    out=r_c[:, nct * hc:nct * (hc + 1)],
                    in_=sc_r[nct * hc:nct * (hc + 1), :].rearrange("a b -> b a"),
                )

            def emit_pd(hc):
                """delta.T rows [hc*512:(hc+1)*512] = (M.T @ Wv.T) + r x bvc,
                drained fp8 to one AR row-half. bf16 matmuls (fp8 M too lossy)."""
                ar_dst = ar_inA if hc == 0 else ar_inB
                for hb in range(4):
                    dst = arst.tile([P, DIM], f8, tag="ar", name=f"ds{hc}_{hb}")
                    hg = hc * 4 + hb        # global h-tile index
                    for oc in range(2):
                        pd = psum.tile([P, 512], f32, tag="ps",
                                       name=f"pd{hc}_{hb}_{oc}")
                        for i in range(NT):
                            nc.tensor.matmul(
                                pd[:],
                                mb[:, i, hg * P:(hg + 1) * P],
                                wv16[:, i, oc * 512:(oc + 1) * 512],
                                start=(i == 0), stop=(i == NT - 1),
                            )
                        pt = tmp.tile([P, 512], f32, tag="kv",
                                      name=f"pt{hc}_{hb}_{oc}")
                        nc.scalar.activation(
                            pt[:], pd[:], AF.Copy, scale=pd_drain_scale
                        )
                        nc.vector.scalar_tensor_tensor(
                            dst[:, oc * 512:(oc + 1) * 512],
                            bvcp_b[:, oc * 512:(oc + 1) * 512],
                            r_c[:, hg:hg + 1],
                            pt[:],
                            op0=ALU.mult,
                            op1=ALU.add,
                        )
                    nc.sync.dma_start(
                        out=ar_dst[hb * P:(hb + 1) * P, :], in_=dst[:]
                    )

            def emit_prsT(bc):
                """prsT[b] = sum_h et8[h, b] for one 512-col chunk."""
                pp = psmall.tile([1, 512], f32, tag="pl", name=f"pp{bc}")
                for j in range(NT // 2):
                    nc.tensor.matmul(
                        pp[:],
                        ones8[:, 0:2, 0:1],
                        et8[:, 2 * j:2 * j + 2, bc * 512:(bc + 1) * 512],
                        start=(j == 0), stop=(j == NT // 2 - 1),
                        perf_mode=DR,
                    )
                nc.vector.tensor_copy(prsT_sb[0:1, bc * 512:(bc + 1) * 512], pp[:])
                nc.sync.dma_start(
                    out=sc_prs[nct * bc:nct * (bc + 1), :],
                    in_=prsT_sb[0:1, bc * 512:(bc + 1) * 512],
                )

            def emit_recip():
                nc.sync.dma_start(
                    out=recip_c[:, :], in_=sc_prs[:, :].rearrange("a b -> b a")
                )
                nc.vector.reciprocal(recip_c[:], recip_c[:])
                nc.vector.tensor_scalar_mul(recip_c[:], recip_c[:], fin_recip_scale)

            def emit_fin():
                """out = (et8.T @ wn) * recip + wfb, fp8 DR, h-contraction
                accumulates over both AR halves."""
                for t in range(nbt):
                    o_st = ost.tile([P, DIM], bf16, tag="os", name=f"os{t}")
                    for oc in range(2):
                        po = psum.tile([P, 512], f32, tag="ps", name=f"po{t}_{oc}")
                        for j in range(NT // 2):
                            wn = wn8a if j < 2 else wn8b
                            jj = j % 2
                            nc.tensor.matmul(
                                po[:],
                                et8[:, 2 * j:2 * j + 2, t * P:(t + 1) * P],
                                wn[:, 2 * jj:2 * jj + 2, oc * 512:(oc + 1) * 512],
                                start=(j == 0), stop=(j == NT // 2 - 1),
                                perf_mode=DR,
                            )
                        nc.vector.scalar_tensor_tensor(
                            o_st[:, oc * 512:(oc + 1) * 512],
                            po[:],
                            recip_c[:, t:t + 1],
                            wfb_b[:, oc * 512:(oc + 1) * 512],
                            op0=ALU.mult,
                            op1=ALU.add,
                        )
                    eng = nc.sync if t % 2 == 0 else nc.scalar
                    eng.dma_start(
                        out=out_h[t * P:(t + 1) * P, :], in_=o_st[:]
                    )

            # ---- schedule ----
            emit_q([0])           # q chunk 0 warms up the PE
            emit_lrT(0)
            emit_lrT(1)
            emit_k(range(0, nbt // 2), 0)
            emit_lrT(2)
            emit_lrT(3)
            emit_k(range(nbt // 2, nbt), 0)
            emit_m(0)
            emit_pd(0)
            nc.gpsimd.collective_compute(
                "AllReduce",
                mybir.AluOpType.add,
                replica_groups=[list(range(n_cores))],
                ins=[ar_inA[:, :]],
                outs=[ar_outA[:, :]],
            )
            for hb in range(4):
                nc.scalar.dma_start(
                    out=wn8a[:, hb, :], in_=ar_outA[hb * P:(hb + 1) * P, :]
                )
            emit_k(range(nbt), 1)
            emit_m(1)
            emit_pd(1)
            nc.gpsimd.collective_compute(
                "AllReduce",
                mybir.AluOpType.add,
                replica_groups=[list(range(n_cores))],
                ins=[ar_inB[:, :]],
                outs=[ar_outB[:, :]],
            )
            for hb in range(4):
                nc.scalar.dma_start(
                    out=wn8b[:, hb, :], in_=ar_outB[hb * P:(hb + 1) * P, :]
                )
            emit_prsT(0)
            for bc in range(1, nbc):
                emit_q([bc])
                emit_prsT(bc)
            emit_recip()
            emit_fin()

    nc.compile()
    return nc


def _tile_rows(a, tile_rows=P):
    """[R, C] -> [128, (R//128)*C] partition-major contiguous layout."""
    r, c = a.shape
    nt = r // tile_rows
    return np.ascontiguousarray(
        a.reshape(nt, tile_rows, c).transpose(1, 0, 2).reshape(tile_rows, nt * c)
    )


def _host_prep(x, W_slow_w, W_slow_b, W_fast_b, b_core, n_cores):
    """Shard + pre-transpose + cast inputs; returns per-core input maps."""
    Wk = W_slow_w[:DIM]
    Wv = W_slow_w[DIM:2 * DIM]
    Wq = W_slow_w[2 * DIM:3 * DIM]
    wlr = W_slow_w[3 * DIM]

    WkT8 = np.clip(np.ascontiguousarray(Wk.T) * SWK, -240.0, 240.0).astype(F8E4)
    wk8A = _tile_rows(np.ascontiguousarray(WkT8[:, :512]))
    wk8B = _tile_rows(np.ascontiguousarray(WkT8[:, 512:]))
    wv16 = _tile_rows(np.ascontiguousarray(Wv.T).astype(BF16))
    wq8 = _tile_rows(
        np.clip(np.ascontiguousarray(Wq.T) * SWQ, -240.0, 240.0).astype(F8E4)
    )
    wlr8 = np.clip(wlr[:512] * SWL, -240.0, 240.0).astype(F8E4)
    wlr16 = (wlr[512:] * (SX * SWL)).astype(BF16)

    bk = (W_slow_b[:DIM] * SKP).astype(np.float32)
    b_total = float(b_core * n_cores)
    bvcp = ((W_slow_b[DIM:2 * DIM] - W_fast_b) * (SAR / (b_total * SG))).astype(
        np.float32
    )
    bqs = (W_slow_b[2 * DIM:3 * DIM] - SHIFT).astype(np.float32)
    blr = np.ascontiguousarray(W_slow_b[3 * DIM:3 * DIM + 1]).astype(np.float32)
    wfb = np.ascontiguousarray(W_fast_b).astype(np.float32)

    in_maps = []
    for c in range(n_cores):
        xs = x[c * b_core:(c + 1) * b_core, :]
        xT = np.ascontiguousarray(xs.T)
        xT8 = np.clip(xT * SX, -240.0, 240.0).astype(F8E4)
        xT8a = _tile_rows(np.ascontiguousarray(xT8[:, :b_core // 2]))
        xT8b = _tile_rows(np.ascontiguousarray(xT8[:, b_core // 2:]))
        xT16 = _tile_rows(np.ascontiguousarray(xT[512:]).astype(BF16))
        xn8 = _tile_rows(
            np.clip(xs * SX, -240.0, 240.0).astype(F8E4)
        )
        in_maps.append({
            "xT8a": xT8a, "xT8b": xT8b, "xT16": xT16, "xn8": xn8,
            "wk8A": wk8A, "wk8B": wk8B, "wq8": wq8, "wv16": wv16,
            "wlr8": wlr8, "wlr16": wlr16,
            "bk": bk, "bqs": bqs, "blr": blr, "bvcp": bvcp, "wfb": wfb,
        })
    return in_maps


_PROGRAM_CACHE = {}


def _get_program(b_core, n_cores=N_CORES):
    key = (b_core, n_cores)
    if key not in _PROGRAM_CACHE:
        _PROGRAM_CACHE[key] = _build_program(b_core, n_cores)
    return _PROGRAM_CACHE[key]


def _run_device(x, W_slow_w, W_slow_b, W_fast_b, trace=False):
    from concourse.bass_utils import run_bass_kernel_spmd

    b_core = x.shape[0] // N_CORES
    nc = _get_program(b_core)
    in_maps = _host_prep(x, W_slow_w, W_slow_b, W_fast_b, b_core, N_CORES)
    res = run_bass_kernel_spmd(nc, in_maps, list(range(N_CORES)), trace=trace)
    out = np.concatenate([res.results[c]["out"] for c in range(N_CORES)], axis=0)
    return out.astype(np.float32), res


def _reference_numpy(x, W_slow_w, W_slow_b, W_fast_w, W_fast_b):
    """Exact fallback (only used if W_fast_w != 0, which the spec never produces)."""
    x = x.astype(np.float64)
    s = x @ W_slow_w.astype(np.float64).T + W_slow_b.astype(np.float64)
    k = s[:, :DIM]
    v = s[:, DIM:2 * DIM]
    q = s[:, 2 * DIM:3 * DIM]
    lr = 1.0 / (1.0 + np.exp(-s[:, -1:]))
    ek = np.exp(k - k.max(axis=1, keepdims=True))
    ak = ek / ek.sum(axis=1, keepdims=True)
    v_bar = ak @ W_fast_w.astype(np.float64).T + W_fast_b.astype(np.float64)
    sigk = 1.0 / (1.0 + np.exp(-k))
    delta = (lr * (v - v_bar)).T @ sigk / x.shape[0]
    w_new = W_fast_w.astype(np.float64) + delta
    eq = np.exp(q - q.max(axis=1, keepdims=True))
    aq = eq / eq.sum(axis=1, keepdims=True)
    return (aq @ w_new.T + W_fast_b.astype(np.float64)).astype(np.float32)


def kernel(x, W_slow_w, W_slow_b, W_fast_w, W_fast_b):
    x = np.asarray(x)
    W_slow_w = np.asarray(W_slow_w)
    W_slow_b = np.asarray(W_slow_b)
    W_fast_w = np.asarray(W_fast_w)
    W_fast_b = np.asarray(W_fast_b)
    if np.any(W_fast_w):
        # Spec guarantees W_fast_w == 0; exact fallback for generality.
        return _reference_numpy(x, W_slow_w, W_slow_b, W_fast_w, W_fast_b)
    out, _ = _run_device(x, W_slow_w, W_slow_b, W_fast_b, trace=False)
    return out


# revision 5
# speedup vs baseline: 1.6889x; 1.6889x over previous
"""DeltaNet fused kernel for 8 TRN2 NeuronCores (Bass/Tile), fp8-hybrid v3.

Math (reference, with W_fast_w == 0 so v_bar == W_fast_b):
    s  = x @ W_slow_w.T + W_slow_b            [B, 3073]
    k  = s[:, :1024]; v = s[:, 1024:2048]; q = s[:, 2048:3072]
    lr = sigmoid(s[:, 3072])
    delta[o,h] = sum_b (lr*(v - wfb))[b,o] * sigmoid(k)[b,h] / B
    out = softmax(q) @ delta.T + wfb

Restructured to eliminate the v projection (v = x @ Wv.T + bv):
    g  = lr * sigmoid(k)                      [B, H]
    M  = x.T @ g                              [I, H]   (per-core partial)
    r  = sum_b g[b, :]                        [H]
    delta.T = (M.T @ Wv.T + r x (bv - wfb)) / B        [H, O]  (AllReduced)
    out = softmax(q) @ delta.T + wfb

v3 changes vs v2 (275us):
  * k contraction fully fp8-DR (validated: rel err unchanged at 1.11e-2 sim);
    drops wk16 + the 64 bf16 k matmuls.
  * delta.T is split by H-ROWS (not O-columns): pd half A only needs the
    h<512 part of g/M, so AR-A triggers at ~80us instead of ~177us and both
    AllReduces hide under k-B/M-B/pd-B + the q chunks.
  * lr multiply (g8 = sgk * lr) moved from Scalar to Pool so the k drain is
    one V add + one S sigmoid + one Pool mul -- no engine exceeds PE pace.
  * Host-side pre-tiled contiguous DMA layouts: one dma_start per big
    tensor with 4-16KB per-partition rows (was ~858B descriptors at
    ~206GB/s aggregate).
  * fin accumulates over both wn halves (h-contraction), runs last.
"""

import os
import sys

for _p in ("/opt/trn_rl_repo", "/root/.axon_site/_ro/trn_rl_repo"):
    if os.path.isdir(_p) and _p not in sys.path:
        sys.path.append(_p)

import numpy as np
import ml_dtypes

BF16 = ml_dtypes.bfloat16
F8E4 = ml_dtypes.float8_e4m3     # TRN fp8e4: max normal +-240

N_CORES = 8
B_FULL = 16384
DIM = 1024          # dim_in == dim_out == dim_hidden
P = 128
NT = DIM // P       # 8 tiles along any 1024 dim

SX = 16.0           # x fp8 scale             (|x|max 5.4  -> 87)
SWQ = 512.0         # Wq fp8 scale            (|w|max .16  -> 80)
SWK = 512.0         # Wk fp8 scale
SKP = SX * SWK      # k psum scale
SWL = 512.0         # wlr fp8 scale (i < 512 half)
SG = 128.0          # g fp8 scale             (g in (0,1)  -> <128)
SAR = 4096.0        # delta fp8 scale         (|delta|max .043 -> 176)
SHIFT = 3.0         # exp shift               (max q+bq 7.63 -> et < 103)


def _build_program(b_core: int, n_cores: int = N_CORES):
    """Build the SPMD Bass program (same program on every core)."""
    import concourse.bass as bass
    import concourse.mybir as mybir
    import concourse.tile as tile
    from concourse import bacc

    f32 = mybir.dt.float32
    bf16 = mybir.dt.bfloat16
    f8 = mybir.dt.float8e4
    AF = mybir.ActivationFunctionType
    ALU = mybir.AluOpType
    DR = mybir.MatmulPerfMode.DoubleRow

    nbt = b_core // P               # b-tiles per core (16)
    nbc = b_core // 512             # 512-wide b-chunks (4)
    nct = 512 // P                  # b-tiles per chunk (4)
    assert b_core % 1024 == 0

    nc = bacc.Bacc(
        "TRN2",
        target_bir_lowering=False,
        debug=False,
        num_devices=n_cores,
    )

    # ---- kernel I/O (host pre-tiled, fully contiguous) ----
    xT8a_h = nc.dram_tensor("xT8a", [P, NT * (b_core // 2)], f8, kind="ExternalInput")
    xT8b_h = nc.dram_tensor("xT8b", [P, NT * (b_core // 2)], f8, kind="ExternalInput")
    xn8_h = nc.dram_tensor("xn8", [P, nbt * DIM], f8, kind="ExternalInput")
    xT16_h = nc.dram_tensor("xT16", [P, 4 * b_core], bf16, kind="ExternalInput")
    wq8_h = nc.dram_tensor("wq8", [P, NT * DIM], f8, kind="ExternalInput")
    wk8A_h = nc.dram_tensor("wk8A", [P, NT * 512], f8, kind="ExternalInput")
    wk8B_h = nc.dram_tensor("wk8B", [P, NT * 512], f8, kind="ExternalInput")
    wv16_h = nc.dram_tensor("wv16", [P, NT * DIM], bf16, kind="ExternalInput")
    wlr8_h = nc.dram_tensor("wlr8", [512], f8, kind="ExternalInput")
    wlr16_h = nc.dram_tensor("wlr16", [512], bf16, kind="ExternalInput")
    bk_h = nc.dram_tensor("bk", [DIM], f32, kind="ExternalInput")     # bk * SKP
    bqs_h = nc.dram_tensor("bqs", [DIM], f32, kind="ExternalInput")   # bq - SHIFT
    blr_h = nc.dram_tensor("blr", [1], f32, kind="ExternalInput")
    # (bv - wfb) * SAR / (b_total * SG): outer-product operand for the drain
    bvcp_h = nc.dram_tensor("bvcp", [DIM], f32, kind="ExternalInput")
    wfb_h = nc.dram_tensor("wfb", [DIM], f32, kind="ExternalInput")
    out_h = nc.dram_tensor("out", [b_core, DIM], bf16, kind="ExternalOutput")

    pd_drain_scale = SAR / (float(b_core * n_cores) * SX * SG)
    fin_recip_scale = 1.0 / SAR
    q_act_scale = 1.0 / (SX * SWQ)
    k_act_scale = 1.0 / SKP
    lr_act_scale = 1.0 / (SX * SWL)

    with tile.TileContext(nc) as tc:
        with (
            tc.tile_pool(name="persist", bufs=1) as persist,
            tc.tile_pool(name="psum", bufs=6, space="PSUM") as psum,
            tc.tile_pool(name="psmall", bufs=2, space="PSUM") as psmall,
            tc.tile_pool(name="tmp", bufs=4) as tmp,
            tc.tile_pool(name="sgt", bufs=4) as sgt,
            tc.tile_pool(name="ost", bufs=4) as ost,
            tc.tile_pool(name="arst", bufs=3) as arst,
            tc.tile_pool(name="dram", bufs=1, space="DRAM") as dram,
        ):
            # ---- persistent SBUF tensors ----
            wq8 = persist.tile([P, NT, DIM], f8, name="wq8")
            xT8a = persist.tile([P, NT, b_core // 2], f8, name="xT8a")
            xT8b = persist.tile([P, NT, b_core // 2], f8, name="xT8b")
            wk8A = persist.tile([P, NT, 512], f8, name="wk8A")
            wk8B = persist.tile([P, NT, 512], f8, name="wk8B")
            # [P, 4, 16] with data in col 0: DR LDWEIGHTS needs pair step %16==0
            wlr8 = persist.tile([P, 4, 16], f8, name="wlr8")
            wlr16 = persist.tile([P, 4, 1], bf16, name="wlr16")
            xT16 = persist.tile([P, 4, b_core], bf16, name="xT16")
            xn8 = persist.tile([P, nbt, DIM], f8, name="xn8")
            wv16 = persist.tile([P, NT, DIM], bf16, name="wv16")
            g8 = persist.tile([P, nbt, DIM], f8, name="g8")
            et8 = persist.tile([P, NT, b_core], f8, name="et8")
            mb = persist.tile([P, NT, DIM], bf16, name="mb")
            wn8a = persist.tile([P, 4, DIM], f8, name="wn8a")
            wn8b = persist.tile([P, 4, DIM], f8, name="wn8b")
            bk_b = persist.tile([P, DIM], f32, name="bk_b")
            wfb_b = persist.tile([P, DIM], f32, name="wfb_b")
            bvcp_b = persist.tile([P, DIM], f32, name="bvcp_b")
            bq_c = persist.tile([P, NT], f32, name="bq_c")
            blr_c = persist.tile([P, 1], f32, name="blr_c")
            lr_c = persist.tile([P, nbt], f32, name="lr_c")
            r_c = persist.tile([P, NT], f32, name="r_c")
            recip_c = persist.tile([P, nbt], f32, name="recip_c")
            lrT_sb = persist.tile([1, b_core], f32, name="lrT_sb")
            rT_sb = persist.tile([1, DIM], f32, name="rT_sb")
            prsT_sb = persist.tile([1, b_core], f32, name="prsT_sb")
            ones8 = persist.tile([P, 2, 16], f8, name="ones8")
            ones_row = persist.tile([1, P], f32, name="ones_row")

            # ---- DRAM: AllReduce bounce (h-row halves) + transpose scratch ----
            ar_inA = dram.tile([512, DIM], f8, name="ar_inA")
            ar_inB = dram.tile([512, DIM], f8, name="ar_inB")
            ar_outA = dram.tile([512, DIM], f8, name="ar_outA", addr_space="Shared")
            ar_outB = dram.tile([512, DIM], f8, name="ar_outB", addr_space="Shared")
            sc_lr = dram.tile([nbt, P], f32, name="sc_lr")
            sc_r = dram.tile([NT, P], f32, name="sc_r")
            sc_prs = dram.tile([nbt, P], f32, name="sc_prs")
            warm_in = dram.tile([P, 512], f8, name="warm_in")
            warm_out = dram.tile([P, 512], f8, name="warm_out", addr_space="Shared")

            nc.vector.memset(ones8[:], 1.0)
            nc.vector.memset(ones_row[:], 1.0)

            # warm up the collective stream: the first collective pays a
            # ~50us barrier/setup cost; burn it on a tiny AllReduce that
            # runs concurrently with the startup DMAs.
            wtmp = tmp.tile([P, 512], f8, tag="kv", name="wtmp")
            nc.vector.memset(wtmp[:], 0.0)
            nc.gpsimd.dma_start(out=warm_in[:, :], in_=wtmp[:])
            nc.gpsimd.collective_compute(
                "AllReduce",
                mybir.AluOpType.add,
                replica_groups=[list(range(n_cores))],
                ins=[warm_in[:, :]],
                outs=[warm_out[:, :]],
            )

            # ---- small DMAs (gpsimd queue) ----
            nc.gpsimd.dma_start(
                out=bq_c[:],
                in_=bass.AP(tensor=bqs_h, offset=0, ap=[[1, P], [P, NT]]),
            )
            nc.gpsimd.dma_start(
                out=blr_c[:],
                in_=bass.AP(tensor=blr_h, offset=0, ap=[[0, P], [1, 1]]),
            )
            for i in range(4):
                nc.gpsimd.dma_start(
                    out=wlr8[:, i, 0:1],
                    in_=bass.AP(tensor=wlr8_h, offset=i * P, ap=[[1, P], [P, 1]]),
                )
                nc.gpsimd.dma_start(
                    out=wlr16[:, i, :],
                    in_=bass.AP(tensor=wlr16_h, offset=i * P, ap=[[1, P], [P, 1]]),
                )
            # bias broadcasts across partitions via K=1 ones-matmuls
            for bi, (bias_dst, bias_src) in enumerate(
                ((bk_b, bk_h), (wfb_b, wfb_h), (bvcp_b, bvcp_h))
            ):
                for c in range(2):
                    brow = tmp.tile([1, 512], f32, tag="br", name=f"br{bi}_{c}")
                    nc.gpsimd.dma_start(
                        out=brow[:],
                        in_=bass.AP(tensor=bias_src, offset=c * 512,
                                    ap=[[0, 1], [1, 512]]),
                    )
                    pb = psum.tile([P, 512], f32, tag="ps", name=f"pb{bi}_{c}")
                    nc.tensor.matmul(
                        pb[:], ones_row[:, :], brow[:], start=True, stop=True
                    )
                    nc.vector.tensor_copy(bias_dst[:, c * 512:(c + 1) * 512], pb[:])

            # ---- bulk DMAs: single contiguous transfers per tensor ----
            # sync queue: q/k stream operands then natural-layout x
            nc.sync.dma_start(out=xT8a[:, :, :], in_=xT8a_h[:, :])
            nc.sync.dma_start(out=wk8A[:, :, :], in_=wk8A_h[:, :])
            nc.sync.dma_start(
                out=xn8[:, 0:nbt // 2, :], in_=xn8_h[:, 0:(nbt // 2) * DIM]
            )
            nc.sync.dma_start(out=xT8b[:, :, :], in_=xT8b_h[:, :])
            nc.sync.dma_start(out=wk8B[:, :, :], in_=wk8B_h[:, :])
            nc.sync.dma_start(
                out=xn8[:, nbt // 2:, :], in_=xn8_h[:, (nbt // 2) * DIM:]
            )
            # scalar queue: wq8 first (q0 warmup), then lr operand, then wv
            nc.scalar.dma_start(out=wq8[:, :, :], in_=wq8_h[:, :])
            nc.scalar.dma_start(
                out=xT16[:, 0:2, :], in_=xT16_h[:, 0:2 * b_core]
            )
            nc.scalar.dma_start(
                out=xT16[:, 2:4, :], in_=xT16_h[:, 2 * b_core:]
            )
            nc.scalar.dma_start(out=wv16[:, :, :], in_=wv16_h[:, :])

            def xt8_lhs(t, j2):
                """fp8 xT lhsT pair slice for global b-tile t, i-pair j2."""
                src = xT8a if t < nbt // 2 else xT8b
                tc_ = t % (nbt // 2)
                return src[:, 2 * j2:2 * j2 + 2, tc_ * P:(tc_ + 1) * P]

            def emit_q(chunks):
                """et8 = exp(qT + bq - SHIFT), transposed layout [h, b]. fp8 DR."""
                for bc in chunks:
                    src = xT8a if bc < nbc // 2 else xT8b
                    lo = (bc % (nbc // 2)) * 512
                    for hb in range(NT):
                        pq = psum.tile([P, 512], f32, tag="ps", name=f"pq{bc}_{hb}")
                        for j in range(NT // 2):
                            nc.tensor.matmul(
                                pq[:],
                                wq8[:, 2 * j:2 * j + 2, hb * P:(hb + 1) * P],
                                src[:, 2 * j:2 * j + 2, lo:lo + 512],
                                start=(j == 0), stop=(j == NT // 2 - 1),
                                perf_mode=DR,
                            )
                        nc.scalar.activation(
                            et8[:, hb, bc * 512:(bc + 1) * 512], pq[:], AF.Exp,
                            bias=bq_c[:, hb:hb + 1], scale=q_act_scale,
                        )

            def emit_lrT(bc):
                """lr_c[p, 4bc+j] = sigmoid(x @ wlr + blr) * SG for one
                512-col chunk, via a transposed [1, 512] matmul + bounce."""
                src8 = xT8a if bc < nbc // 2 else xT8b
                lo = (bc % (nbc // 2)) * 512
                pl = psmall.tile([1, 512], f32, tag="pl", name=f"plr{bc}")
                for m in range(2):
                    nc.tensor.matmul(
                        pl[:],
                        wlr8[:, 2 * m:2 * m + 2, 0:1],
                        src8[:, 2 * m:2 * m + 2, lo:lo + 512],
                        start=(m == 0), stop=False,
                        perf_mode=DR,
                    )
                for i in range(4):
                    nc.tensor.matmul(
                        pl[:],
                        wlr16[:, i, 0:1],
                        xT16[:, i, bc * 512:(bc + 1) * 512],
                        start=False, stop=(i == 3),
                    )
                nc.scalar.activation(
                    lrT_sb[0:1, bc * 512:(bc + 1) * 512], pl[:], AF.Sigmoid,
                    bias=blr_c[0:1, 0:1], scale=lr_act_scale,
                )
                nc.gpsimd.dma_start(
                    out=sc_lr[nct * bc:nct * (bc + 1), :],
                    in_=lrT_sb[0:1, bc * 512:(bc + 1) * 512],
                )
                nc.gpsimd.dma_start(
                    out=lr_c[:, nct * bc:nct * (bc + 1)],
                    in_=sc_lr[nct * bc:nct * (bc + 1), :].rearrange("a b -> b a"),
                )
                nc.vector.tensor_scalar_mul(
                    lr_c[:, nct * bc:nct * (bc + 1)],
                    lr_c[:, nct * bc:nct * (bc + 1)], SG,
                )

            def emit_k(tiles, hc):
                """g8[:, t, hc half] = lr * sigmoid(k), natural layout [b, h].
                Contraction fully fp8-DR. Drain: V add bias, S sigmoid,
                Pool multiply by lr."""
                wk = wk8A if hc == 0 else wk8B
                for t in tiles:
                    pk = psum.tile([P, 512], f32, tag="ps", name=f"pk{t}_{hc}")
                    for j2 in range(NT // 2):
                        nc.tensor.matmul(
                            pk[:],
                            xt8_lhs(t, j2),
                            wk[:, 2 * j2:2 * j2 + 2, :],
                            start=(j2 == 0), stop=(j2 == NT // 2 - 1),
                            perf_mode=DR,
                        )
                    ktmp = tmp.tile([P, 512], f32, tag="kv", name=f"kt{t}_{hc}")
                    nc.vector.tensor_add(
                        ktmp[:], pk[:], bk_b[:, hc * 512:(hc + 1) * 512]
                    )
                    sgk = sgt.tile([P, 512], bf16, tag="sg", name=f"sg{t}_{hc}")
                    nc.scalar.activation(sgk[:], ktmp[:], AF.Sigmoid,
                                         scale=k_act_scale)
                    # lr multiply alternates S/V so neither falls behind PE
                    if t % 2 == 0:
                        nc.scalar.activation(
                            g8[:, t, hc * 512:(hc + 1) * 512], sgk[:], AF.Copy,
                            scale=lr_c[:, t:t + 1],
                        )
                    else:
                        nc.vector.tensor_scalar_mul(
                            g8[:, t, hc * 512:(hc + 1) * 512], sgk[:],
                            lr_c[:, t:t + 1],
                        )

            def emit_m(hc):
                """mb[:, :, hc half] = x.T @ g (per-core partial), fp8 DR;
                then rT half = ones.T @ g via [1, 512] matmuls + bounce."""
                for ib in range(NT):
                    pm = psum.tile([P, 512], f32, tag="ps", name=f"pm{hc}_{ib}")
                    for bp in range(nbt // 2):
                        nc.tensor.matmul(
                            pm[:],
                            xn8[:, 2 * bp:2 * bp + 2, ib * P:(ib + 1) * P],
                            g8[:, 2 * bp:2 * bp + 2, hc * 512:(hc + 1) * 512],
                            start=(bp == 0), stop=(bp == nbt // 2 - 1),
                            perf_mode=DR,
                        )
                    nc.vector.tensor_copy(
                        mb[:, ib, hc * 512:(hc + 1) * 512], pm[:]
                    )
                pr = psmall.tile([1, 512], f32, tag="pl", name=f"pr{hc}")
                for bp in range(nbt // 2):
                    nc.tensor.matmul(
                        pr[:],
                        ones8[:, 0:2, 0:1],
                        g8[:, 2 * bp:2 * bp + 2, hc * 512:(hc + 1) * 512],
                        start=(bp == 0), stop=(bp == nbt // 2 - 1),
                        perf_mode=DR,
                    )
                nc.vector.tensor_copy(
                    rT_sb[0:1, hc * 512:(hc + 1) * 512], pr[:]
                )
                nc.gpsimd.dma_start(
                    out=sc_r[nct * hc:nct * (hc + 1), :],
                    in_=rT_sb[0:1, hc * 512:(hc + 1) * 512],
                )
                nc.gpsimd.dma_start(
                    out=r_c[:, nct * hc:nct * (hc + 1)],
                    in_=sc_r[nct * hc:nct * (hc + 1), :].rearrange("a b -> b a"),
                )

            def emit_pd(hc):
                """delta.T rows [hc*512:(hc+1)*512] = (M.T @ Wv.T) + r x bvc,
                drained fp8 to one AR row-half. bf16 matmuls (fp8 M too lossy)."""
                ar_dst = ar_inA if hc == 0 else ar_inB
                for hb in range(4):
                    dst = arst.tile([P, DIM], f8, tag="ar", name=f"ds{hc}_{hb}")
                    hg = hc * 4 + hb        # global h-tile index
                    for oc in range(2):
                        pd = psum.tile([P, 512], f32, tag="ps",
                                       name=f"pd{hc}_{hb}_{oc}")
                        for i in range(NT):
                            nc.tensor.matmul(
                                pd[:],
                                mb[:, i, hg * P:(hg + 1) * P],
                                wv16[:, i, oc * 512:(oc + 1) * 512],
                                start=(i == 0), stop=(i == NT - 1),
                            )
                        pt = tmp.tile([P, 512], f32, tag="kv",
                                      name=f"pt{hc}_{hb}_{oc}")
                        nc.scalar.activation(
                            pt[:], pd[:], AF.Copy, scale=pd_drain_scale
                        )
                        nc.vector.scalar_tensor_tensor(
                            dst[:, oc * 512:(oc + 1) * 512],
                            bvcp_b[:, oc * 512:(oc + 1) * 512],
                            r_c[:, hg:hg + 1],
                            pt[:],
                            op0=ALU.mult,
                            op1=ALU.add,
                        )
                    nc.sync.dma_start(
                        out=ar_dst[hb * P:(hb + 1) * P, :], in_=dst[:]
                    )

            def emit_prsT(bc):
                """prsT[b] = sum_h et8[h, b] for one 512-col chunk."""
                pp = psmall.tile([1, 512], f32, tag="pl", name=f"pp{bc}")
                for j in range(NT // 2):
                    nc.tensor.matmul(
                        pp[:],
                        ones8[:, 0:2, 0:1],
                        et8[:, 2 * j:2 * j + 2, bc * 512:(bc + 1) * 512],
                        start=(j == 0), stop=(j == NT // 2 - 1),
                        perf_mode=DR,
                    )
                nc.vector.tensor_copy(prsT_sb[0:1, bc * 512:(bc + 1) * 512], pp[:])
                nc.sync.dma_start(
                    out=sc_prs[nct * bc:nct * (bc + 1), :],
                    in_=prsT_sb[0:1, bc * 512:(bc + 1) * 512],
                )

            def emit_recip():
                nc.sync.dma_start(
                    out=recip_c[:, :], in_=sc_prs[:, :].rearrange("a b -> b a")
                )
                nc.vector.reciprocal(recip_c[:], recip_c[:])
                nc.vector.tensor_scalar_mul(recip_c[:], recip_c[:], fin_recip_scale)

            def emit_fin():
                """out = (et8.T @ wn) * recip + wfb, fp8 DR, h-contraction
                accumulates over both AR halves."""
                for t in range(nbt):
                    o_st = ost.tile([P, DIM], bf16, tag="os", name=f"os{t}")
                    for oc in range(2):
                        po = psum.tile([P, 512], f32, tag="ps", name=f"po{t}_{oc}")
                        for j in range(NT // 2):
                            wn = wn8a if j < 2 else wn8b
                            jj = j % 2
                            nc.tensor.matmul(
                                po[:],
                                et8[:, 2 * j:2 * j + 2, t * P:(t + 1) * P],
                                wn[:, 2 * jj:2 * jj + 2, oc * 512:(oc + 1) * 512],
                                start=(j == 0), stop=(j == NT // 2 - 1),
                                perf_mode=DR,
                            )
                        nc.vector.scalar_tensor_tensor(
                            o_st[:, oc * 512:(oc + 1) * 512],
                            po[:],
                            recip_c[:, t:t + 1],
                            wfb_b[:, oc * 512:(oc + 1) * 512],
                            op0=ALU.mult,
                            op1=ALU.add,
                        )
                    eng = nc.sync if t % 2 == 0 else nc.scalar
                    eng.dma_start(
                        out=out_h[t * P:(t + 1) * P, :], in_=o_st[:]
                    )

            # ---- schedule ----
            emit_q([0])           # q chunk 0 warms up the PE
            emit_lrT(0)
            emit_lrT(1)
            emit_k(range(0, nbt // 2), 0)
            emit_lrT(2)
            emit_lrT(3)
            emit_k(range(nbt // 2, nbt), 0)
            emit_m(0)
            emit_pd(0)
            nc.gpsimd.collective_compute(
                "AllReduce",
                mybir.AluOpType.add,
                replica_groups=[list(range(n_cores))],
                ins=[ar_inA[:, :]],
                outs=[ar_outA[:, :]],
            )
            for hb in range(4):
                nc.scalar.dma_start(
                    out=wn8a[:, hb, :], in_=ar_outA[hb * P:(hb + 1) * P, :]
                )
            emit_k(range(nbt), 1)
            emit_m(1)
            emit_pd(1)
            nc.gpsimd.collective_compute(
                "AllReduce",
                mybir.AluOpType.add,
                replica_groups=[list(range(n_cores))],
                ins=[ar_inB[:, :]],
                outs=[ar_outB[:, :]],
            )
            for hb in range(4):
                nc.scalar.dma_start(
                    out=wn8b[:, hb, :], in_=ar_outB[hb * P:(hb + 1) * P, :]
                )
            emit_prsT(0)
            for bc in range(1, nbc):
                emit_q([bc])
                emit_prsT(bc)
            emit_recip()
            emit_fin()

    nc.compile()
    return nc


def _tile_rows(a, tile_rows=P):
    """[R, C] -> [128, (R//128)*C] partition-major contiguous layout."""
    r, c = a.shape
    nt = r // tile_rows
    return np.ascontiguousarray(
        a.reshape(nt, tile_rows, c).transpose(1, 0, 2).reshape(tile_rows, nt * c)
    )


def _host_prep(x, W_slow_w, W_slow_b, W_fast_b, b_core, n_cores):
    """Shard + pre-transpose + cast inputs; returns per-core input maps."""
    Wk = W_slow_w[:DIM]
    Wv = W_slow_w[DIM:2 * DIM]
    Wq = W_slow_w[2 * DIM:3 * DIM]
    wlr = W_slow_w[3 * DIM]

    WkT8 = np.clip(np.ascontiguousarray(Wk.T) * SWK, -240.0, 240.0).astype(F8E4)
    wk8A = _tile_rows(np.ascontiguousarray(WkT8[:, :512]))
    wk8B = _tile_rows(np.ascontiguousarray(WkT8[:, 512:]))
    wv16 = _tile_rows(np.ascontiguousarray(Wv.T).astype(BF16))
    wq8 = _tile_rows(
        np.clip(np.ascontiguousarray(Wq.T) * SWQ, -240.0, 240.0).astype(F8E4)
    )
    wlr8 = np.clip(wlr[:512] * SWL, -240.0, 240.0).astype(F8E4)
    wlr16 = (wlr[512:] * (SX * SWL)).astype(BF16)

    bk = (W_slow_b[:DIM] * SKP).astype(np.float32)
    b_total = float(b_core * n_cores)
    bvcp = ((W_slow_b[DIM:2 * DIM] - W_fast_b) * (SAR / (b_total * SG))).astype(
        np.float32
    )
    bqs = (W_slow_b[2 * DIM:3 * DIM] - SHIFT).astype(np.float32)
    blr = np.ascontiguousarray(W_slow_b[3 * DIM:3 * DIM + 1]).astype(np.float32)
    wfb = np.ascontiguousarray(W_fast_b).astype(np.float32)

    in_maps = []
    for c in range(n_cores):
        xs = x[c * b_core:(c + 1) * b_core, :]
        xT = np.ascontiguousarray(xs.T)
        xT8 = np.clip(xT * SX, -240.0, 240.0).astype(F8E4)
        xT8a = _tile_rows(np.ascontiguousarray(xT8[:, :b_core // 2]))
        xT8b = _tile_rows(np.ascontiguousarray(xT8[:, b_core // 2:]))
        xT16 = _tile_rows(np.ascontiguousarray(xT[512:]).astype(BF16))
        xn8 = _tile_rows(
            np.clip(xs * SX, -240.0, 240.0).astype(F8E4)
        )
        in_maps.append({
            "xT8a": xT8a, "xT8b": xT8b, "xT16": xT16, "xn8": xn8,
            "wk8A": wk8A, "wk8B": wk8B, "wq8": wq8, "wv16": wv16,
            "wlr8": wlr8, "wlr16": wlr16,
            "bk": bk, "bqs": bqs, "blr": blr, "bvcp": bvcp, "wfb": wfb,
        })
    return in_maps


_PROGRAM_CACHE = {}


def _get_program(b_core, n_cores=N_CORES):
    key = (b_core, n_cores)
    if key not in _PROGRAM_CACHE:
        _PROGRAM_CACHE[key] = _build_program(b_core, n_cores)
    return _PROGRAM_CACHE[key]


def _run_device(x, W_slow_w, W_slow_b, W_fast_b, trace=False):
    from concourse.bass_utils import run_bass_kernel_spmd

    b_core = x.shape[0] // N_CORES
    nc = _get_program(b_core)
    in_maps = _host_prep(x, W_slow_w, W_slow_b, W_fast_b, b_core, N_CORES)
    res = run_bass_kernel_spmd(nc, in_maps, list(range(N_CORES)), trace=trace)
    out = np.concatenate([res.results[c]["out"] for c in range(N_CORES)], axis=0)
    return out.astype(np.float32), res


def _reference_numpy(x, W_slow_w, W_slow_b, W_fast_w, W_fast_b):
    """Exact fallback (only used if W_fast_w != 0, which the spec never produces)."""
    x = x.astype(np.float64)
    s = x @ W_slow_w.astype(np.float64).T + W_slow_b.astype(np.float64)
    k = s[:, :DIM]
    v = s[:, DIM:2 * DIM]
    q = s[:, 2 * DIM:3 * DIM]
    lr = 1.0 / (1.0 + np.exp(-s[:, -1:]))
    ek = np.exp(k - k.max(axis=1, keepdims=True))
    ak = ek / ek.sum(axis=1, keepdims=True)
    v_bar = ak @ W_fast_w.astype(np.float64).T + W_fast_b.astype(np.float64)
    sigk = 1.0 / (1.0 + np.exp(-k))
    delta = (lr * (v - v_bar)).T @ sigk / x.shape[0]
    w_new = W_fast_w.astype(np.float64) + delta
    eq = np.exp(q - q.max(axis=1, keepdims=True))
    aq = eq / eq.sum(axis=1, keepdims=True)
    return (aq @ w_new.T + W_fast_b.astype(np.float64)).astype(np.float32)


def kernel(x, W_slow_w, W_slow_b, W_fast_w, W_fast_b):
    x = np.asarray(x)
    W_slow_w = np.asarray(W_slow_w)
    W_slow_b = np.asarray(W_slow_b)
    W_fast_w = np.asarray(W_fast_w)
    W_fast_b = np.asarray(W_fast_b)
    if np.any(W_fast_w):
        # Spec guarantees W_fast_w == 0; exact fallback for generality.
        return _reference_numpy(x, W_slow_w, W_slow_b, W_fast_w, W_fast_b)
    out, _ = _run_device(x, W_slow_w, W_slow_b, W_fast_b, trace=False)
    return out


# revision 21
# speedup vs baseline: 1.8374x; 1.0879x over previous
"""DeltaNet fused kernel for 8 TRN2 NeuronCores (Bass/Tile), fp8-hybrid v3.

Math (reference, with W_fast_w == 0 so v_bar == W_fast_b):
    s  = x @ W_slow_w.T + W_slow_b            [B, 3073]
    k  = s[:, :1024]; v = s[:, 1024:2048]; q = s[:, 2048:3072]
    lr = sigmoid(s[:, 3072])
    delta[o,h] = sum_b (lr*(v - wfb))[b,o] * sigmoid(k)[b,h] / B
    out = softmax(q) @ delta.T + wfb

Restructured to eliminate the v projection (v = x @ Wv.T + bv):
    g  = lr * sigmoid(k)                      [B, H]
    M  = x.T @ g                              [I, H]   (per-core partial)
    r  = sum_b g[b, :]                        [H]
    delta.T = (M.T @ Wv.T + r x (bv - wfb)) / B        [H, O]  (AllReduced)
    out = softmax(q) @ delta.T + wfb

v3 changes vs v2 (275us):
  * k contraction fully fp8-DR (validated: rel err unchanged at 1.11e-2 sim);
    drops wk16 + the 64 bf16 k matmuls.
  * delta.T is split by H-ROWS (not O-columns): pd half A only needs the
    h<512 part of g/M, so AR-A triggers at ~80us instead of ~177us and both
    AllReduces hide under k-B/M-B/pd-B + the q chunks.
  * lr multiply (g8 = sgk * lr) moved from Scalar to Pool so the k drain is
    one V add + one S sigmoid + one Pool mul -- no engine exceeds PE pace.
  * Host-side pre-tiled contiguous DMA layouts: one dma_start per big
    tensor with 4-16KB per-partition rows (was ~858B descriptors at
    ~206GB/s aggregate).
  * fin accumulates over both wn halves (h-contraction), runs last.
"""

import os
import sys

for _p in ("/opt/trn_rl_repo", "/root/.axon_site/_ro/trn_rl_repo"):
    if os.path.isdir(_p) and _p not in sys.path:
        sys.path.append(_p)

import numpy as np
import ml_dtypes

BF16 = ml_dtypes.bfloat16
F8E4 = ml_dtypes.float8_e4m3     # TRN fp8e4: max normal +-240

N_CORES = 8
B_FULL = 16384
DIM = 1024          # dim_in == dim_out == dim_hidden
P = 128
NT = DIM // P       # 8 tiles along any 1024 dim

SX = 16.0           # x fp8 scale             (|x|max 5.4  -> 87)
SWQ = 512.0         # Wq fp8 scale            (|w|max .16  -> 80)
SWK = 512.0         # Wk fp8 scale
SKP = SX * SWK      # k psum scale
SWL = 512.0         # wlr fp8 scale (i < 512 half)
SG = 128.0          # g fp8 scale             (g in (0,1)  -> <128)
SAR = 4096.0        # delta fp8 scale         (|delta|max .043 -> 176)
SHIFT = 3.0         # exp shift               (max q+bq 7.63 -> et < 103)


def _build_program(b_core: int, n_cores: int = N_CORES):
    """Build the SPMD Bass program (same program on every core)."""
    import concourse.bass as bass
    import concourse.mybir as mybir
    import concourse.tile as tile
    from concourse import bacc

    f32 = mybir.dt.float32
    bf16 = mybir.dt.bfloat16
    f8 = mybir.dt.float8e4
    AF = mybir.ActivationFunctionType
    ALU = mybir.AluOpType
    DR = mybir.MatmulPerfMode.DoubleRow

    nbt = b_core // P               # b-tiles per core (16)
    nbc = b_core // 512             # 512-wide b-chunks (4)
    nct = 512 // P                  # b-tiles per chunk (4)
    assert b_core % 1024 == 0

    nc = bacc.Bacc(
        "TRN2",
        target_bir_lowering=False,
        debug=False,
        num_devices=n_cores,
    )

    # ---- kernel I/O (host pre-tiled, fully contiguous) ----
    xT8a_h = nc.dram_tensor("xT8a", [P, NT * (b_core // 2)], f8, kind="ExternalInput")
    xT8b_h = nc.dram_tensor("xT8b", [P, NT * (b_core // 2)], f8, kind="ExternalInput")
    xn8_h = nc.dram_tensor("xn8", [P, nbt * DIM], f8, kind="ExternalInput")
    wq8_h = nc.dram_tensor("wq8", [P, NT * DIM], f8, kind="ExternalInput")
    wk8A_h = nc.dram_tensor("wk8A", [P, NT * 512], f8, kind="ExternalInput")
    wk8B_h = nc.dram_tensor("wk8B", [P, NT * 512], f8, kind="ExternalInput")
    wv16_h = nc.dram_tensor("wv16", [P, NT * DIM], bf16, kind="ExternalInput")
    # wl8(1024) | res8(1024): wlr fp8 + error-feedback residual, one row
    wlrow_h = nc.dram_tensor("wlrow", [1, 2048], f8, kind="ExternalInput")
    bk_h = nc.dram_tensor("bk", [DIM], f32, kind="ExternalInput")     # bk * SKP
    bqs_h = nc.dram_tensor("bqs", [DIM], f32, kind="ExternalInput")   # bq - SHIFT
    blr_h = nc.dram_tensor("blr", [1], f32, kind="ExternalInput")
    # (bv - wfb) * SAR / (b_total * SG): outer-product operand for the drain
    bvcp_h = nc.dram_tensor("bvcp", [DIM], f32, kind="ExternalInput")
    wfb_h = nc.dram_tensor("wfb", [DIM], f32, kind="ExternalInput")
    out_h = nc.dram_tensor("out", [b_core, DIM], bf16, kind="ExternalOutput")

    pd_drain_scale = SAR / (float(b_core * n_cores) * SX * SG)
    fin_recip_scale = 1.0 / SAR
    q_act_scale = 1.0 / (SX * SWQ)
    k_act_scale = 1.0 / SKP
    lr_act_scale = 1.0 / (SX * SWL)

    with tile.TileContext(nc) as tc:
        with (
            tc.tile_pool(name="persist", bufs=1) as persist,
            tc.tile_pool(name="psum", bufs=6, space="PSUM") as psum,
            tc.tile_pool(name="psmall", bufs=2, space="PSUM") as psmall,
            tc.tile_pool(name="tmp", bufs=4) as tmp,
            tc.tile_pool(name="sgt", bufs=4) as sgt,
            tc.tile_pool(name="ost", bufs=4) as ost,
            tc.tile_pool(name="arst", bufs=3) as arst,
            tc.tile_pool(name="dram", bufs=1, space="DRAM") as dram,
        ):
            # ---- persistent SBUF tensors ----
            wq8 = persist.tile([P, NT, DIM], f8, name="wq8")
            xT8a = persist.tile([P, NT, b_core // 2], f8, name="xT8a")
            xT8b = persist.tile([P, NT, b_core // 2], f8, name="xT8b")
            wk8A = persist.tile([P, NT, 512], f8, name="wk8A")
            wk8B = persist.tile([P, NT, 512], f8, name="wk8B")
            # [P, 8, 16] with data in col 0: DR LDWEIGHTS needs pair step %16==0
            wlr8 = persist.tile([P, 8, 16], f8, name="wlr8")
            wlres8 = persist.tile([P, 8, 16], f8, name="wlres8")
            wlrow = persist.tile([1, 2048], f8, name="wlrow")
            xn8 = persist.tile([P, nbt, DIM], f8, name="xn8")
            wv16 = persist.tile([P, NT, DIM], bf16, name="wv16")
            g8 = persist.tile([P, nbt, DIM], f8, name="g8")
            et8 = persist.tile([P, NT, b_core], f8, name="et8")
            mb = persist.tile([P, NT, DIM], bf16, name="mb")
            wn8a = persist.tile([P, 4, DIM], f8, name="wn8a")
            wn8b = persist.tile([P, 4, DIM], f8, name="wn8b")
            bk_b = persist.tile([P, DIM], f32, name="bk_b")
            wfb_b = persist.tile([P, DIM], f32, name="wfb_b")
            bvcp_b = persist.tile([P, DIM], f32, name="bvcp_b")
            bq_c = persist.tile([P, NT], f32, name="bq_c")
            blr_c = persist.tile([P, 1], f32, name="blr_c")
            lr_c = persist.tile([P, nbt], f32, name="lr_c")
            r_c = persist.tile([P, NT], f32, name="r_c")
            recip_c = persist.tile([P, nbt], f32, name="recip_c")
            lrT_sb = persist.tile([1, b_core], f32, name="lrT_sb")
            rT_sb = persist.tile([1, DIM], f32, name="rT_sb")
            prsT_sb = persist.tile([1, b_core], f32, name="prsT_sb")
            ones8 = persist.tile([P, 2, 16], f8, name="ones8")
            ones_row = persist.tile([1, P], f32, name="ones_row")

            # ---- DRAM: AllReduce bounce (h-row halves) + transpose scratch ----
            ar_inA = dram.tile([512, DIM], f8, name="ar_inA")
            ar_inB = dram.tile([512, DIM], f8, name="ar_inB")
            ar_outA = dram.tile([512, DIM], f8, name="ar_outA", addr_space="Shared")
            ar_outB = dram.tile([512, DIM], f8, name="ar_outB", addr_space="Shared")
            sc_lr = dram.tile([nbt, P], f32, name="sc_lr")
            sc_r = dram.tile([NT, P], f32, name="sc_r")
            sc_prs = dram.tile([nbt, P], f32, name="sc_prs")
            warm_in = dram.tile([P, 512], f8, name="warm_in")
            warm_out = dram.tile([P, 512], f8, name="warm_out", addr_space="Shared")

            nc.vector.memset(ones8[:], 1.0)
            nc.vector.memset(ones_row[:], 1.0)

            # warm up the collective stream: the first collective pays a
            # ~50us barrier/setup cost; burn it on a tiny AllReduce that
            # runs concurrently with the startup DMAs.
            wtmp = tmp.tile([P, 512], f8, tag="kv", name="wtmp")
            nc.vector.memset(wtmp[:], 0.0)
            nc.gpsimd.dma_start(out=warm_in[:, :], in_=wtmp[:])
            nc.gpsimd.collective_compute(
                "AllReduce",
                mybir.AluOpType.add,
                replica_groups=[list(range(n_cores))],
                ins=[warm_in[:, :]],
                outs=[warm_out[:, :]],
            )

            # ---- small DMAs (gpsimd queue) ----
            nc.gpsimd.dma_start(
                out=bq_c[:],
                in_=bass.AP(tensor=bqs_h, offset=0, ap=[[1, P], [P, NT]]),
            )
            nc.gpsimd.dma_start(
                out=blr_c[:],
                in_=bass.AP(tensor=blr_h, offset=0, ap=[[0, P], [1, 1]]),
            )
            # ---- bulk DMAs: single contiguous transfers per tensor ----
            # single-descriptor rows first: wlr row + bias rows
            nc.sync.dma_start(out=wlrow[:, :], in_=wlrow_h[:, :])
            brows = {}
            for bi, (bias_dst, bias_src, eng) in enumerate((
                (bk_b, bk_h, nc.sync), (wfb_b, wfb_h, nc.scalar),
                (bvcp_b, bvcp_h, nc.scalar),
            )):
                for c in range(2):
                    brow = tmp.tile([1, 512], f32, tag="br", bufs=6,
                                    name=f"br{bi}_{c}")
                    eng.dma_start(
                        out=brow[:],
                        in_=bass.AP(tensor=bias_src, offset=c * 512,
                                    ap=[[0, 1], [1, 512]]),
                    )
                    brows[(bi, c)] = brow
            # sync queue: q/k stream operands then second half of natural x
            nc.sync.dma_start(out=xT8a[:, :, :], in_=xT8a_h[:, :])
            nc.sync.dma_start(out=wk8A[:, :, :], in_=wk8A_h[:, :])
            nc.sync.dma_start(out=xT8b[:, :, :], in_=xT8b_h[:, :])
            nc.sync.dma_start(out=wk8B[:, :, :], in_=wk8B_h[:, :])
            nc.sync.dma_start(
                out=xn8[:, nbt // 2:, :], in_=xn8_h[:, (nbt // 2) * DIM:]
            )
            # scalar queue: wq8 first (q0 warmup), xn first half, then wv
            nc.scalar.dma_start(out=wq8[:, :, :], in_=wq8_h[:, :])
            nc.scalar.dma_start(
                out=xn8[:, 0:nbt // 2, :], in_=xn8_h[:, 0:(nbt // 2) * DIM]
            )
            nc.scalar.dma_start(out=wv16[:, :, :], in_=wv16_h[:, :])

            # ---- wlr column vectors via PE transpose (row -> [128,1] psum) ----
            for i in range(NT):
                for src_off, dst in ((0, wlr8), (DIM, wlres8)):
                    ptr = psmall.tile([P, 16], f8, tag="pl", name=f"ptr{src_off}_{i}")
                    nc.tensor.transpose(
                        ptr[:, 0:1],
                        wlrow[0:1, src_off + i * P:src_off + (i + 1) * P],
                        ones8[0:1, 0, 0:1],
                    )
                    nc.vector.tensor_copy(dst[:, i, 0:1], ptr[:, 0:1])

            # ---- bias broadcasts across partitions via K=1 ones-matmuls ----
            for bi, (bias_dst, _, _) in enumerate((
                (bk_b, bk_h, None), (wfb_b, wfb_h, None), (bvcp_b, bvcp_h, None),
            )):
                for c in range(2):
                    pb = psum.tile([P, 512], f32, tag="ps", name=f"pb{bi}_{c}")
                    nc.tensor.matmul(
                        pb[:], ones_row[:, :], brows[(bi, c)][:],
                        start=True, stop=True
                    )
                    nc.vector.tensor_copy(bias_dst[:, c * 512:(c + 1) * 512], pb[:])

            def xt8_lhs(t, j2):
                """fp8 xT lhsT pair slice for global b-tile t, i-pair j2."""
                src = xT8a if t < nbt // 2 else xT8b
                tc_ = t % (nbt // 2)
                return src[:, 2 * j2:2 * j2 + 2, tc_ * P:(tc_ + 1) * P]

            def emit_q(chunks, hbs=None):
                """et8 = exp(qT + bq - SHIFT), transposed layout [h, b]. fp8 DR."""
                for bc in chunks:
                    src = xT8a if bc < nbc // 2 else xT8b
                    lo = (bc % (nbc // 2)) * 512
                    for hb in (range(NT) if hbs is None else hbs):
                        pq = psum.tile([P, 512], f32, tag="ps", name=f"pq{bc}_{hb}")
                        for j in range(NT // 2):
                            nc.tensor.matmul(
                                pq[:],
                                wq8[:, 2 * j:2 * j + 2, hb * P:(hb + 1) * P],
                                src[:, 2 * j:2 * j + 2, lo:lo + 512],
                                start=(j == 0), stop=(j == NT // 2 - 1),
                                perf_mode=DR,
                            )
                        nc.scalar.activation(
                            et8[:, hb, bc * 512:(bc + 1) * 512], pq[:], AF.Exp,
                            bias=bq_c[:, hb:hb + 1], scale=q_act_scale,
                        )

            def emit_lrT(bc):
                """lr_c[p, 4bc+j] = sigmoid(x @ wlr + blr) * SG for one
                512-col chunk. wlr = wl8 + res8 (fp8 error feedback), both
                groups accumulate into one transposed [1, 512] psum."""
                src8 = xT8a if bc < nbc // 2 else xT8b
                lo = (bc % (nbc // 2)) * 512
                pl = psmall.tile([1, 512], f32, tag="pl", name=f"plr{bc}")
                for wi, w in enumerate((wlr8, wlres8)):
                    for m in range(4):
                        nc.tensor.matmul(
                            pl[:],
                            w[:, 2 * m:2 * m + 2, 0:1],
                            src8[:, 2 * m:2 * m + 2, lo:lo + 512],
                            start=(wi == 0 and m == 0), stop=(wi == 1 and m == 3),
                            perf_mode=DR,
                        )
                nc.scalar.activation(
                    lrT_sb[0:1, bc * 512:(bc + 1) * 512], pl[:], AF.Sigmoid,
                    bias=blr_c[0:1, 0:1], scale=lr_act_scale,
                )
                nc.gpsimd.dma_start(
                    out=sc_lr[nct * bc:nct * (bc + 1), :],
                    in_=lrT_sb[0:1, bc * 512:(bc + 1) * 512],
                )
                nc.gpsimd.dma_start(
                    out=lr_c[:, nct * bc:nct * (bc + 1)],
                    in_=sc_lr[nct * bc:nct * (bc + 1), :].rearrange("a b -> b a"),
                )
                nc.vector.tensor_scalar_mul(
                    lr_c[:, nct * bc:nct * (bc + 1)],
                    lr_c[:, nct * bc:nct * (bc + 1)], SG,
                )

            def emit_k(tiles, hc):
                """g8[:, t, hc half] = lr * sigmoid(k), natural layout [b, h].
                Contraction fully fp8-DR. Drain: V add bias, S sigmoid,
                Pool multiply by lr."""
                wk = wk8A if hc == 0 else wk8B
                for t in tiles:
                    pk = psum.tile([P, 512], f32, tag="ps", name=f"pk{t}_{hc}")
                    for j2 in range(NT // 2):
                        nc.tensor.matmul(
                            pk[:],
                            xt8_lhs(t, j2),
                            wk[:, 2 * j2:2 * j2 + 2, :],
                            start=(j2 == 0), stop=(j2 == NT // 2 - 1),
                            perf_mode=DR,
                        )
                    ktmp = tmp.tile([P, 512], f32, tag="kv", name=f"kt{t}_{hc}")
                    nc.vector.tensor_add(
                        ktmp[:], pk[:], bk_b[:, hc * 512:(hc + 1) * 512]
                    )
                    sgk = sgt.tile([P, 512], bf16, tag="sg", name=f"sg{t}_{hc}")
                    nc.scalar.activation(sgk[:], ktmp[:], AF.Sigmoid,
                                         scale=k_act_scale)
                    # lr multiply alternates S/V so neither falls behind PE
                    if t % 2 == 0:
                        nc.scalar.activation(
                            g8[:, t, hc * 512:(hc + 1) * 512], sgk[:], AF.Copy,
                            scale=lr_c[:, t:t + 1],
                        )
                    else:
                        nc.vector.tensor_scalar_mul(
                            g8[:, t, hc * 512:(hc + 1) * 512], sgk[:],
                            lr_c[:, t:t + 1],
                        )

            def emit_m(hc):
                """mb[:, :, hc half] = x.T @ g (per-core partial), fp8 DR;
                then rT half = ones.T @ g via [1, 512] matmuls + bounce."""
                for ib in range(NT):
                    pm = psum.tile([P, 512], f32, tag="ps", name=f"pm{hc}_{ib}")
                    for bp in range(nbt // 2):
                        nc.tensor.matmul(
                            pm[:],
                            xn8[:, 2 * bp:2 * bp + 2, ib * P:(ib + 1) * P],
                            g8[:, 2 * bp:2 * bp + 2, hc * 512:(hc + 1) * 512],
                            start=(bp == 0), stop=(bp == nbt // 2 - 1),
                            perf_mode=DR,
                        )
                    nc.vector.tensor_copy(
                        mb[:, ib, hc * 512:(hc + 1) * 512], pm[:]
                    )
                pr = psmall.tile([1, 512], f32, tag="pl", name=f"pr{hc}")
                for bp in range(nbt // 2):
                    nc.tensor.matmul(
                        pr[:],
                        ones8[:, 0:2, 0:1],
                        g8[:, 2 * bp:2 * bp + 2, hc * 512:(hc + 1) * 512],
                        start=(bp == 0), stop=(bp == nbt // 2 - 1),
                        perf_mode=DR,
                    )
                nc.vector.tensor_copy(
                    rT_sb[0:1, hc * 512:(hc + 1) * 512], pr[:]
                )
                nc.gpsimd.dma_start(
                    out=sc_r[nct * hc:nct * (hc + 1), :],
                    in_=rT_sb[0:1, hc * 512:(hc + 1) * 512],
                )
                nc.gpsimd.dma_start(
                    out=r_c[:, nct * hc:nct * (hc + 1)],
                    in_=sc_r[nct * hc:nct * (hc + 1), :].rearrange("a b -> b a"),
                )

            def emit_pd(hc):
                """delta.T rows [hc*512:(hc+1)*512] = (M.T @ Wv.T) + r x bvc,
                drained fp8 to one AR row-half. bf16 matmuls (fp8 M too lossy)."""
                ar_dst = ar_inA if hc == 0 else ar_inB
                for hb in range(4):
                    dst = arst.tile([P, DIM], f8, tag="ar", name=f"ds{hc}_{hb}")
                    hg = hc * 4 + hb        # global h-tile index
                    for oc in range(2):
                        pd = psum.tile([P, 512], f32, tag="ps",
                                       name=f"pd{hc}_{hb}_{oc}")
                        for i in range(NT):
                            nc.tensor.matmul(
                                pd[:],
                                mb[:, i, hg * P:(hg + 1) * P],
                                wv16[:, i, oc * 512:(oc + 1) * 512],
                                start=(i == 0), stop=(i == NT - 1),
                            )
                        pt = tmp.tile([P, 512], f32, tag="kv",
                                      name=f"pt{hc}_{hb}_{oc}")
                        nc.scalar.activation(
                            pt[:], pd[:], AF.Copy, scale=pd_drain_scale
                        )
                        nc.vector.scalar_tensor_tensor(
                            dst[:, oc * 512:(oc + 1) * 512],
                            bvcp_b[:, oc * 512:(oc + 1) * 512],
                            r_c[:, hg:hg + 1],
                            pt[:],
                            op0=ALU.mult,
                            op1=ALU.add,
                        )
                    nc.sync.dma_start(
                        out=ar_dst[hb * P:(hb + 1) * P, :], in_=dst[:]
                    )

            def emit_prsT(bc):
                """prsT[b] = sum_h et8[h, b] for one 512-col chunk."""
                pp = psmall.tile([1, 512], f32, tag="pl", name=f"pp{bc}")
                for j in range(NT // 2):
                    nc.tensor.matmul(
                        pp[:],
                        ones8[:, 0:2, 0:1],
                        et8[:, 2 * j:2 * j + 2, bc * 512:(bc + 1) * 512],
                        start=(j == 0), stop=(j == NT // 2 - 1),
                        perf_mode=DR,
                    )
                nc.vector.tensor_copy(prsT_sb[0:1, bc * 512:(bc + 1) * 512], pp[:])
                nc.sync.dma_start(
                    out=sc_prs[nct * bc:nct * (bc + 1), :],
                    in_=prsT_sb[0:1, bc * 512:(bc + 1) * 512],
                )

            def emit_recip():
                nc.sync.dma_start(
                    out=recip_c[:, :], in_=sc_prs[:, :].rearrange("a b -> b a")
                )
                nc.vector.reciprocal(recip_c[:], recip_c[:])
                nc.vector.tensor_scalar_mul(recip_c[:], recip_c[:], fin_recip_scale)

            def emit_fin():
                """out = (et8.T @ wn) * recip + wfb, fp8 DR, h-contraction
                accumulates over both AR halves."""
                for t in range(nbt):
                    o_st = ost.tile([P, DIM], bf16, tag="os", name=f"os{t}")
                    for oc in range(2):
                        po = psum.tile([P, 512], f32, tag="ps", name=f"po{t}_{oc}")
                        for j in range(NT // 2):
                            wn = wn8a if j < 2 else wn8b
                            jj = j % 2
                            nc.tensor.matmul(
                                po[:],
                                et8[:, 2 * j:2 * j + 2, t * P:(t + 1) * P],
                                wn[:, 2 * jj:2 * jj + 2, oc * 512:(oc + 1) * 512],
                                start=(j == 0), stop=(j == NT // 2 - 1),
                                perf_mode=DR,
                            )
                        nc.vector.scalar_tensor_tensor(
                            o_st[:, oc * 512:(oc + 1) * 512],
                            po[:],
                            recip_c[:, t:t + 1],
                            wfb_b[:, oc * 512:(oc + 1) * 512],
                            op0=ALU.mult,
                            op1=ALU.add,
                        )
                    eng = nc.sync if t % 2 == 0 else nc.scalar
                    eng.dma_start(
                        out=out_h[t * P:(t + 1) * P, :], in_=o_st[:]
                    )

            # ---- schedule ----
            emit_q([0], hbs=[0, 1])   # small q warmup; rest after AR-B
            emit_lrT(0)
            emit_lrT(1)
            emit_k(range(0, nbt // 2), 0)
            emit_lrT(2)
            emit_lrT(3)
            emit_k(range(nbt // 2, nbt), 0)
            emit_m(0)
            emit_pd(0)
            nc.gpsimd.collective_compute(
                "AllReduce",
                mybir.AluOpType.add,
                replica_groups=[list(range(n_cores))],
                ins=[ar_inA[:, :]],
                outs=[ar_outA[:, :]],
            )
            for hb in range(4):
                nc.scalar.dma_start(
                    out=wn8a[:, hb, :], in_=ar_outA[hb * P:(hb + 1) * P, :]
                )
            emit_k(range(nbt), 1)
            emit_m(1)
            emit_pd(1)
            nc.gpsimd.collective_compute(
                "AllReduce",
                mybir.AluOpType.add,
                replica_groups=[list(range(n_cores))],
                ins=[ar_inB[:, :]],
                outs=[ar_outB[:, :]],
            )
            for hb in range(4):
                nc.scalar.dma_start(
                    out=wn8b[:, hb, :], in_=ar_outB[hb * P:(hb + 1) * P, :]
                )
            emit_q([0], hbs=range(2, NT))   # rest of q chunk 0
            emit_prsT(0)
            for bc in range(1, nbc):
                emit_q([bc])
                emit_prsT(bc)
            emit_recip()
            emit_fin()

    nc.compile()
    return nc


def _tile_rows(a, tile_rows=P):
    """[R, C] -> [128, (R//128)*C] partition-major contiguous layout."""
    r, c = a.shape
    nt = r // tile_rows
    return np.ascontiguousarray(
        a.reshape(nt, tile_rows, c).transpose(1, 0, 2).reshape(tile_rows, nt * c)
    )


def _host_prep(x, W_slow_w, W_slow_b, W_fast_b, b_core, n_cores):
    """Shard + pre-transpose + cast inputs; returns per-core input maps."""
    Wk = W_slow_w[:DIM]
    Wv = W_slow_w[DIM:2 * DIM]
    Wq = W_slow_w[2 * DIM:3 * DIM]
    wlr = W_slow_w[3 * DIM]

    WkT8 = np.clip(np.ascontiguousarray(Wk.T) * SWK, -240.0, 240.0).astype(F8E4)
    wk8A = _tile_rows(np.ascontiguousarray(WkT8[:, :512]))
    wk8B = _tile_rows(np.ascontiguousarray(WkT8[:, 512:]))
    wv16 = _tile_rows(np.ascontiguousarray(Wv.T).astype(BF16))
    wq8 = _tile_rows(
        np.clip(np.ascontiguousarray(Wq.T) * SWQ, -240.0, 240.0).astype(F8E4)
    )
    # wlr fp8 with error-feedback residual: wlr*SWL ~= wl8 + res8
    wl8 = np.clip(wlr * SWL, -240.0, 240.0).astype(F8E4)
    res8 = (wlr * SWL - wl8.astype(np.float64)).astype(F8E4)
    wlrow = np.concatenate([wl8, res8]).reshape(1, 2048)

    bk = (W_slow_b[:DIM] * SKP).astype(np.float32)
    b_total = float(b_core * n_cores)
    bvcp = ((W_slow_b[DIM:2 * DIM] - W_fast_b) * (SAR / (b_total * SG))).astype(
        np.float32
    )
    bqs = (W_slow_b[2 * DIM:3 * DIM] - SHIFT).astype(np.float32)
    blr = np.ascontiguousarray(W_slow_b[3 * DIM:3 * DIM + 1]).astype(np.float32)
    wfb = np.ascontiguousarray(W_fast_b).astype(np.float32)

    in_maps = []
    for c in range(n_cores):
        xs = x[c * b_core:(c + 1) * b_core, :]
        xT = np.ascontiguousarray(xs.T)
        xT8 = np.clip(xT * SX, -240.0, 240.0).astype(F8E4)
        xT8a = _tile_rows(np.ascontiguousarray(xT8[:, :b_core // 2]))
        xT8b = _tile_rows(np.ascontiguousarray(xT8[:, b_core // 2:]))
        xn8 = _tile_rows(
            np.clip(xs * SX, -240.0, 240.0).astype(F8E4)
        )
        in_maps.append({
            "xT8a": xT8a, "xT8b": xT8b, "xn8": xn8,
            "wk8A": wk8A, "wk8B": wk8B, "wq8": wq8, "wv16": wv16,
            "wlrow": wlrow,
            "bk": bk, "bqs": bqs, "blr": blr, "bvcp": bvcp, "wfb": wfb,
        })
    return in_maps


_PROGRAM_CACHE = {}


def _get_program(b_core, n_cores=N_CORES):
    key = (b_core, n_cores)
    if key not in _PROGRAM_CACHE:
        _PROGRAM_CACHE[key] = _build_program(b_core, n_cores)
    return _PROGRAM_CACHE[key]


def _run_device(x, W_slow_w, W_slow_b, W_fast_b, trace=False):
    from concourse.bass_utils import run_bass_kernel_spmd

    b_core = x.shape[0] // N_CORES
    nc = _get_program(b_core)
    in_maps = _host_prep(x, W_slow_w, W_slow_b, W_fast_b, b_core, N_CORES)
    res = run_bass_kernel_spmd(nc, in_maps, list(range(N_CORES)), trace=trace)
    out = np.concatenate([res.results[c]["out"] for c in range(N_CORES)], axis=0)
    return out.astype(np.float32), res


def _reference_numpy(x, W_slow_w, W_slow_b, W_fast_w, W_fast_b):
    """Exact fallback (only used if W_fast_w != 0, which the spec never produces)."""
    x = x.astype(np.float64)
    s = x @ W_slow_w.astype(np.float64).T + W_slow_b.astype(np.float64)
    k = s[:, :DIM]
    v = s[:, DIM:2 * DIM]
    q = s[:, 2 * DIM:3 * DIM]
    lr = 1.0 / (1.0 + np.exp(-s[:, -1:]))
    ek = np.exp(k - k.max(axis=1, keepdims=True))
    ak = ek / ek.sum(axis=1, keepdims=True)
    v_bar = ak @ W_fast_w.astype(np.float64).T + W_fast_b.astype(np.float64)
    sigk = 1.0 / (1.0 + np.exp(-k))
    delta = (lr * (v - v_bar)).T @ sigk / x.shape[0]
    w_new = W_fast_w.astype(np.float64) + delta
    eq = np.exp(q - q.max(axis=1, keepdims=True))
    aq = eq / eq.sum(axis=1, keepdims=True)
    return (aq @ w_new.T + W_fast_b.astype(np.float64)).astype(np.float32)


def kernel(x, W_slow_w, W_slow_b, W_fast_w, W_fast_b):
    x = np.asarray(x)
    W_slow_w = np.asarray(W_slow_w)
    W_slow_b = np.asarray(W_slow_b)
    W_fast_w = np.asarray(W_fast_w)
    W_fast_b = np.asarray(W_fast_b)
    if np.any(W_fast_w):
        # Spec guarantees W_fast_w == 0; exact fallback for generality.
        return _reference_numpy(x, W_slow_w, W_slow_b, W_fast_w, W_fast_b)
    out, _ = _run_device(x, W_slow_w, W_slow_b, W_fast_b, trace=False)
    return out
